# revision 1
# baseline (speedup 1.0000x reference)
"""Trainium2 Bass kernel for nn_CrossTalk (segment scatter-add -> tridiag mix -> gather).

Full (unsharded) inputs in, full output out. Internally shards the wavelength
axis of flux across 8 NeuronCores (512 rows each), and on each core runs a
Tile/Bass kernel that processes four 128-row chunks:

  per 128x7200 chunk (rows = wavelengths on partitions, positions on free axis):
    G1   gpsimd.ap_gather   : sort flux columns by segment id (host-computed perm)
    ACT  scaled copy        : S *= (1 - 2*eta)
    DVE  tensor_tensor_scan : segmented prefix-sum (mask kills state at run starts)
                              -> binned sums sit at run-end columns of B
    G2   gpsimd.ap_gather   : scatter run-end values to a per-tile-padded natural
                              bin layout N (empty bins / pads index B's zero col)
    DVE  tt_add + stt       : tridiagonal mix M = C + eta' * (L + R)
    G3   gpsimd.ap_gather   : gather M back to original positions
    DMA  out

All index metadata (sort permutation, run ends, segment ids) is tiny host-side
precompute from tile_idx/fib_idx; the flux data is only touched on-device.
"""

import os
import sys

import numpy as np

for _p in ("/opt/trn_rl_repo", "/root/.axon_site/_ro/trn_rl_repo"):
    if os.path.isdir(_p) and _p not in sys.path:
        sys.path.insert(0, _p)

import ml_dtypes  # noqa: E402

import concourse.bacc as bacc  # noqa: E402
import concourse.mybir as mybir  # noqa: E402
from concourse.tile import TileContext  # noqa: E402
from concourse.bass_utils import run_bass_kernel_spmd  # noqa: E402

# Problem geometry (fixed by the harness spec).
N_WAVE = 4096
N_TILES = 12
N_FIBRES = 600
N_POS = N_TILES * N_FIBRES          # 7200
N_CORES = 8
ROWS_PER_CORE = N_WAVE // N_CORES   # 512
P = 128                             # SBUF partitions
N_CHUNKS = ROWS_PER_CORE // P       # 4
PAD_W = N_FIBRES + 4                # 604: per-tile padded width (zeros at 0,601..603)
NAT_W = N_TILES * PAD_W             # 7248 (multiple of 16)

F32 = mybir.dt.float32
BF16 = mybir.dt.bfloat16
I16 = mybir.dt.int16

_PROGRAM_CACHE = {}


def _wrap_idx(flat):
    """ap_gather index layout: idxs[p, s] = flat[s*16 + p], tiled to 128 partitions."""
    flat = np.asarray(flat, np.int64)
    assert flat.size % 16 == 0
    w = flat.reshape(flat.size // 16, 16).T.astype(np.int16)   # [16, S]
    return np.tile(w, (P // 16, 1))                            # [128, S]


def _host_precompute(tile_idx, fib_idx):
    seg = (tile_idx.astype(np.int64) * N_FIBRES + fib_idx.astype(np.int64)).astype(np.int64)
    order = np.argsort(seg, kind="stable")
    sseg = seg[order]

    mask = np.zeros(N_POS, np.float32)
    mask[1:] = (sseg[1:] == sseg[:-1]).astype(np.float32)

    is_end = np.ones(N_POS, bool)
    is_end[:-1] = sseg[1:] != sseg[:-1]
    end_j = np.nonzero(is_end)[0]
    endcol = np.zeros(N_POS, np.int64)          # empty bins -> B column 0 (zero)
    endcol[sseg[end_j]] = end_j + 1             # B has a leading zero column

    nat = np.zeros(NAT_W, np.int64)
    nat.reshape(N_TILES, PAD_W)[:, 1:N_FIBRES + 1] = endcol.reshape(N_TILES, N_FIBRES)

    return {
        "sort_idx": _wrap_idx(order),
        "nat_idx": _wrap_idx(nat),
        "out_idx": _wrap_idx(seg),
        "scan_mask": np.tile(mask.astype(ml_dtypes.bfloat16)[None, :], (P, 1)),
    }


def _build_program():
    key = "v1"
    if key in _PROGRAM_CACHE:
        return _PROGRAM_CACHE[key]

    nc = bacc.Bacc("TRN2", target_bir_lowering=False, debug=False)

    flux_d = nc.dram_tensor("flux", [ROWS_PER_CORE, N_POS], F32, kind="ExternalInput").ap()
    sort_d = nc.dram_tensor("sort_idx", [P, N_POS // 16], I16, kind="ExternalInput").ap()
    nat_d = nc.dram_tensor("nat_idx", [P, NAT_W // 16], I16, kind="ExternalInput").ap()
    oidx_d = nc.dram_tensor("out_idx", [P, N_POS // 16], I16, kind="ExternalInput").ap()
    mask_d = nc.dram_tensor("scan_mask", [P, N_POS], BF16, kind="ExternalInput").ap()
    consts_d = nc.dram_tensor("consts", [P, 2], F32, kind="ExternalInput").ap()
    out_d = nc.dram_tensor("out", [ROWS_PER_CORE, N_POS], F32, kind="ExternalOutput").ap()

    mult = mybir.AluOpType.mult
    add = mybir.AluOpType.add
    Copy = mybir.ActivationFunctionType.Copy

    with TileContext(nc) as tc:
        with (
            tc.tile_pool(name="pers", bufs=1) as pers,
            tc.tile_pool(name="fo", bufs=2) as fo_pool,
            tc.tile_pool(name="sw", bufs=2) as sw_pool,
            tc.tile_pool(name="nn", bufs=1) as n_pool,
            tc.tile_pool(name="mm", bufs=1) as m_pool,
        ):
            # ---- one-time setup ----
            sort_t = pers.tile([P, N_POS // 16], I16, tag="sidx")
            nat_t = pers.tile([P, NAT_W // 16], I16, tag="nidx")
            oidx_t = pers.tile([P, N_POS // 16], I16, tag="oidx")
            mask_t = pers.tile([P, N_POS], BF16, tag="mask")
            consts_t = pers.tile([P, 2], F32, tag="consts")
            nc.sync.dma_start(sort_t[:], sort_d)
            nc.sync.dma_start(nat_t[:], nat_d)
            nc.sync.dma_start(oidx_t[:], oidx_d)
            nc.sync.dma_start(mask_t[:], mask_d)
            nc.sync.dma_start(consts_t[:], consts_d)
            c0_ap = consts_t[:, 0:1]     # 1 - 2*eta
            etap_ap = consts_t[:, 1:2]   # eta / (1 - 2*eta)

            F = {}
            S = {}
            N = {}
            M = {}
            O = {}

            def dma_in(i):
                F[i] = fo_pool.tile([P, N_POS], F32, tag="FO", name=f"F{i}")
                nc.sync.dma_start(F[i][:], flux_d[P * i:P * (i + 1), :])

            def front(i):  # G1 + scale + scan (scan runs in place in S cols 1:)
                S[i] = sw_pool.tile([P, N_POS + 1], F32, tag="SW", name=f"S{i}")
                sv = S[i][:, 1:N_POS + 1]
                nc.gpsimd.ap_gather(sv, F[i][:], sort_t[:],
                                    channels=P, num_elems=N_POS, d=1, num_idxs=N_POS)
                nc.scalar.memzero(S[i][:, 0:1])
                nc.scalar.activation(sv, sv, Copy, scale=c0_ap)
                nc.vector.tensor_tensor_scan(sv, mask_t[:], sv, 0.0, mult, add)

            def middle(i):  # G2 + tridiag mix
                N[i] = n_pool.tile([P, NAT_W], F32, tag="N", name=f"N{i}")
                nc.gpsimd.ap_gather(N[i][:], S[i][:], nat_t[:],
                                    channels=P, num_elems=N_POS + 1, d=1, num_idxs=NAT_W)
                nv = N[i][:].rearrange("p (t w) -> p t w", w=PAD_W)
                T1 = sw_pool.tile([P, N_POS], F32, tag="SW", name=f"T1_{i}")
                t1v = T1[:].rearrange("p (t w) -> p t w", w=N_FIBRES)
                nc.vector.tensor_tensor(
                    t1v, nv[:, :, 0:N_FIBRES], nv[:, :, 2:N_FIBRES + 2], add)
                M[i] = m_pool.tile([P, N_POS], F32, tag="M", name=f"M{i}")
                mv = M[i][:].rearrange("p (t w) -> p t w", w=N_FIBRES)
                nc.vector.scalar_tensor_tensor(
                    mv, t1v, etap_ap, nv[:, :, 1:N_FIBRES + 1], mult, add)

            def back(i):  # G3 + store
                O[i] = fo_pool.tile([P, N_POS], F32, tag="FO", name=f"O{i}")
                nc.gpsimd.ap_gather(O[i][:], M[i][:], oidx_t[:],
                                    channels=P, num_elems=N_POS, d=1, num_idxs=N_POS)
                nc.sync.dma_start(out_d[P * i:P * (i + 1), :], O[i][:])

            # software-pipelined emission: GPSIMD stream per step t is
            # [G1(t), G2(t-1), G3(t-2)] so the (bottleneck) gather engine
            # never sits behind a same-chunk DVE dependency.
            dma_in(0)
            if N_CHUNKS > 1:
                dma_in(1)
            for t in range(N_CHUNKS + 2):
                if t < N_CHUNKS:
                    front(t)
                    if t + 2 < N_CHUNKS:
                        dma_in(t + 2)
                if 0 <= t - 1 < N_CHUNKS:
                    middle(t - 1)
                if 0 <= t - 2 < N_CHUNKS:
                    back(t - 2)

    nc.compile()
    _PROGRAM_CACHE[key] = nc
    return nc


def _run(inputs, trace=False, trace_kwargs=None):
    flux = np.ascontiguousarray(np.asarray(inputs["flux"], dtype=np.float32))
    eta = float(np.asarray(inputs["eta"]).reshape(-1)[0])
    tile_idx = np.asarray(inputs["tile_idx"]).astype(np.int64).reshape(-1)
    fib_idx = np.asarray(inputs["fib_idx"]).astype(np.int64).reshape(-1)

    assert flux.shape == (N_WAVE, N_POS), flux.shape
    assert tile_idx.shape == (N_POS,) and fib_idx.shape == (N_POS,)

    aux = _host_precompute(tile_idx, fib_idx)
    c0 = np.float32(1.0 - 2.0 * eta)
    etap = np.float32(eta / float(c0))
    consts = np.tile(np.array([[c0, etap]], np.float32), (P, 1))

    nc = _build_program()

    shared = {
        "sort_idx": aux["sort_idx"],
        "nat_idx": aux["nat_idx"],
        "out_idx": aux["out_idx"],
        "scan_mask": aux["scan_mask"],
        "consts": consts,
    }
    in_maps = []
    for c in range(N_CORES):
        m = dict(shared)
        m["flux"] = np.ascontiguousarray(
            flux[c * ROWS_PER_CORE:(c + 1) * ROWS_PER_CORE])
        in_maps.append(m)

    kwargs = {}
    if trace:
        kwargs["trace"] = True
        if trace_kwargs:
            kwargs["trace_kwargs"] = trace_kwargs
    res = run_bass_kernel_spmd(nc, in_maps, core_ids=list(range(N_CORES)), **kwargs)
    out = np.concatenate([r["out"] for r in res.results], axis=0)
    return out, res


def kernel(**inputs) -> np.ndarray:
    out, _ = _run(inputs, trace=False)
    return out



# revision 2
# speedup vs baseline: 3.1119x; 3.1119x over previous
"""Trainium2 Bass kernel for nn_CrossTalk (segment scatter-add -> tridiag mix -> gather).

Full (unsharded) inputs in, full output out. Shards the wavelength axis of
flux across 8 NeuronCores (512 rows each). Per core, the 512 rows are packed
as FOUR bf16 lanes interleaved along the free axis ([128, 7200, 4]), so each
gpsimd ap_gather index moves all four row-chunks at once (gather cost is
per-index, not per-byte: 29ns/idx at d=4 vs 28ns/idx at d=1).

Positions are processed in two halves split at the tile-6 boundary of the
sorted order, which (a) fits SBUF and (b) pipelines DVE scan/mix work under
the gpsimd gathers:

  G1h  gpsimd.ap_gather : sort flux columns of half h by segment id
  DVE  tensor_tensor_scan x4 lanes: segmented prefix-sum (strided views)
  G2h  gpsimd.ap_gather : run-end sums -> per-tile padded natural layout
  DVE  tt + stt         : tridiag mix M = C + (eta/c0) * (L + R)
  G3h  gpsimd.ap_gather : gather M back to original positions (output halves)
  ACT  copy scale=c0    : bf16 -> f32, final scale folded in
  DMA  out

Precision: values ride in bf16 (scan state is fp32 internally); measured
rel-err vs the f32 reference ~6e-3 (harness gate 2e-2).
"""

import os
import sys

import numpy as np

for _p in ("/opt/trn_rl_repo", "/root/.axon_site/_ro/trn_rl_repo"):
    if os.path.isdir(_p) and _p not in sys.path:
        sys.path.insert(0, _p)

import ml_dtypes  # noqa: E402

import concourse.bacc as bacc  # noqa: E402
import concourse.mybir as mybir  # noqa: E402
from concourse.tile import TileContext  # noqa: E402
from concourse.bass_utils import run_bass_kernel_spmd  # noqa: E402

# Problem geometry (fixed by the harness spec).
N_WAVE = 4096
N_TILES = 12
N_FIBRES = 600
N_POS = N_TILES * N_FIBRES          # 7200
N_CORES = 8
ROWS_PER_CORE = N_WAVE // N_CORES   # 512
P = 128                             # SBUF partitions
D = 4                               # row-chunks packed per gather payload
PAD_W = N_FIBRES + 4                # 604 per-tile padded width
HALF_TILES = N_TILES // 2           # 6
NAT_H = HALF_TILES * PAD_W          # 3624 natural slots per half
MHALF = HALF_TILES * N_FIBRES       # 3600 compact M columns per half

F32 = mybir.dt.float32
BF16 = mybir.dt.bfloat16
I16 = mybir.dt.int16

_PROGRAM_CACHE = {}


def _ceil(x, m):
    return (x + m - 1) // m * m


def _wrap_idx(flat, num_idxs):
    """ap_gather idx layout: flat list wrapped over 16 partitions, padded so
    the ucode's ceil_div(num_idxs,32) uint32 reads stay in bounds; tiled to
    128 partitions."""
    flat = np.asarray(flat, np.int64)
    assert flat.size == num_idxs
    padded = _ceil(num_idxs, 32)
    if padded != num_idxs:
        flat = np.concatenate([flat, np.zeros(padded - num_idxs, np.int64)])
    w = flat.reshape(padded // 16, 16).T.astype(np.int16)   # [16, S]
    return np.tile(w, (P // 16, 1))                          # [128, S]


def _host_precompute(tile_idx, fib_idx):
    seg = (tile_idx.astype(np.int64) * N_FIBRES + fib_idx.astype(np.int64))
    order = np.argsort(seg, kind="stable")
    sseg = seg[order]

    is_end = np.ones(N_POS, bool)
    is_end[:-1] = sseg[1:] != sseg[:-1]
    end_j = np.nonzero(is_end)[0]

    o6 = int(np.searchsorted(sseg, HALF_TILES * N_FIBRES))  # sorted split
    halves = []
    for h in range(2):
        lo, hi = (0, o6) if h == 0 else (o6, N_POS)
        n = hi - lo
        n_p = _ceil(n, 16)

        sort_flat = np.zeros(n_p, np.int64)
        sort_flat[:n] = order[lo:hi]

        cmask = np.zeros(n_p, np.float32)
        cmask[1:n] = (sseg[lo + 1:hi] == sseg[lo:hi - 1]).astype(np.float32)

        # bin -> local run-end column in S_h (leading zero block at 0)
        endcol = np.zeros(N_TILES * N_FIBRES, np.int64)
        sel = (end_j >= lo) & (end_j < hi)
        ej = end_j[sel]
        endcol[sseg[ej]] = ej - lo + 1

        nat = np.zeros(NAT_H, np.int64)
        nat.reshape(HALF_TILES, PAD_W)[:, 1:N_FIBRES + 1] = (
            endcol.reshape(N_TILES, N_FIBRES)[h * HALF_TILES:(h + 1) * HALF_TILES]
        )

        halves.append({
            "n": n,
            "n_p": n_p,
            "sort_idx": _wrap_idx(sort_flat, n_p),
            "nat_idx": _wrap_idx(nat, NAT_H),
            "out_idx": _wrap_idx(seg[h * MHALF:(h + 1) * MHALF], MHALF),
            "cmask": np.tile(
                cmask.astype(ml_dtypes.bfloat16)[None, :], (P, 1)),
        })
    return halves


def _build_program(n_p):
    key = ("v2", tuple(n_p))
    if key in _PROGRAM_CACHE:
        return _PROGRAM_CACHE[key]

    nc = bacc.Bacc("TRN2", target_bir_lowering=False, debug=False)

    flux_d = nc.dram_tensor("flux", [ROWS_PER_CORE, N_POS], F32,
                            kind="ExternalInput").ap()
    out_d = nc.dram_tensor("out", [ROWS_PER_CORE, N_POS], F32,
                           kind="ExternalOutput").ap()
    consts_d = nc.dram_tensor("consts", [P, 2], F32, kind="ExternalInput").ap()
    sort_d, nat_d, oidx_d, cmask_d = [], [], [], []
    for h in range(2):
        sort_d.append(nc.dram_tensor(
            f"sort_idx{h}", [P, _ceil(n_p[h], 32) // 16], I16,
            kind="ExternalInput").ap())
        nat_d.append(nc.dram_tensor(
            f"nat_idx{h}", [P, _ceil(NAT_H, 32) // 16], I16,
            kind="ExternalInput").ap())
        oidx_d.append(nc.dram_tensor(
            f"out_idx{h}", [P, _ceil(MHALF, 32) // 16], I16,
            kind="ExternalInput").ap())
        cmask_d.append(nc.dram_tensor(
            f"cmask{h}", [P, n_p[h]], BF16, kind="ExternalInput").ap())

    mult = mybir.AluOpType.mult
    add = mybir.AluOpType.add
    Copy = mybir.ActivationFunctionType.Copy

    def lanes(ap_flat, nblk):
        """[P, nblk*D] flat bf16 -> list of D strided [P, nblk] views."""
        v = ap_flat.rearrange("p (j k) -> p j k", k=D)
        return [v[:, :, k:k + 1].squeeze(2) for k in range(D)]

    with TileContext(nc) as tc:
        with (
            tc.tile_pool(name="pers", bufs=1) as pers,
            tc.tile_pool(name="big", bufs=1) as big_pool,
            tc.tile_pool(name="half", bufs=3) as half_pool,
            tc.tile_pool(name="stg", bufs=2) as stg_pool,
        ):
            sort_t, nat_t, oidx_t, cmask_t = [], [], [], []
            for h in range(2):
                st = pers.tile([P, _ceil(n_p[h], 32) // 16], I16, tag=f"si{h}",
                               name=f"sort{h}")
                nt = pers.tile([P, _ceil(NAT_H, 32) // 16], I16, tag=f"ni{h}",
                               name=f"nat{h}")
                ot = pers.tile([P, _ceil(MHALF, 32) // 16], I16, tag=f"oi{h}",
                               name=f"oidx{h}")
                mt = pers.tile([P, n_p[h]], BF16, tag=f"cm{h}", name=f"cmask{h}")
                nc.sync.dma_start(st[:], sort_d[h])
                nc.sync.dma_start(nt[:], nat_d[h])
                nc.sync.dma_start(ot[:], oidx_d[h])
                nc.sync.dma_start(mt[:], cmask_d[h])
                sort_t.append(st)
                nat_t.append(nt)
                oidx_t.append(ot)
                cmask_t.append(mt)
            consts_t = pers.tile([P, 2], F32, tag="consts")
            nc.sync.dma_start(consts_t[:], consts_d)
            c0_ap = consts_t[:, 0:1]     # 1 - 2*eta (final output scale)
            etap_ap = consts_t[:, 1:2]   # eta / (1 - 2*eta)

            # ---- load + pack flux into F [P, N_POS, D] bf16 ----
            F = big_pool.tile([P, N_POS * D], BF16, tag="BIG", name="F")
            for k in range(D):
                for h in range(2):
                    fin = stg_pool.tile([P, MHALF], F32, tag="STG",
                                        name=f"fin{k}{h}")
                    nc.sync.dma_start(
                        fin[:], flux_d[P * k:P * (k + 1),
                                       MHALF * h:MHALF * (h + 1)])
                    dst = lanes(F[:, MHALF * h * D:MHALF * (h + 1) * D],
                                MHALF)[k]
                    nc.scalar.activation(dst, fin[:], Copy)

            S = [None, None]
            N = [None, None]
            M = big_pool.tile([P, N_POS * D], BF16, tag="BIG", name="M")
            O = [None, None]

            def g1(h):
                S[h] = half_pool.tile([P, (1 + n_p[h]) * D], BF16, tag="HALF",
                                      name=f"S{h}")
                nc.vector.memset(S[h][:, 0:D], 0.0)
                nc.gpsimd.ap_gather(S[h][:, D:], F[:], sort_t[h][:],
                                    channels=P, num_elems=N_POS, d=D,
                                    num_idxs=n_p[h])

            def scans(h):
                for lv in lanes(S[h][:, D:], n_p[h]):
                    nc.vector.tensor_tensor_scan(
                        lv, cmask_t[h][:], lv, 0.0, mult, add)

            def g2(h):
                N[h] = half_pool.tile([P, NAT_H * D], BF16, tag="HALF",
                                      name=f"N{h}")
                nc.gpsimd.ap_gather(N[h][:], S[h][:], nat_t[h][:],
                                    channels=P, num_elems=1 + n_p[h], d=D,
                                    num_idxs=NAT_H)

            def mix(h):
                nv = N[h][:].rearrange("p (t w k) -> p t w k", w=PAD_W, k=D)
                mv = M[:, MHALF * h * D:MHALF * (h + 1) * D].rearrange(
                    "p (t w k) -> p t w k", w=N_FIBRES, k=D)
                nc.vector.tensor_tensor(
                    mv, nv[:, :, 0:N_FIBRES, :],
                    nv[:, :, 2:N_FIBRES + 2, :], add)
                nc.vector.scalar_tensor_tensor(
                    mv, mv, etap_ap, nv[:, :, 1:N_FIBRES + 1, :], mult, add)

            def g3(h):
                O[h] = half_pool.tile([P, MHALF * D], BF16, tag="HALF",
                                      name=f"O{h}")
                nc.gpsimd.ap_gather(O[h][:], M[:], oidx_t[h][:],
                                    channels=P, num_elems=N_POS, d=D,
                                    num_idxs=MHALF)

            def out_half(h):
                for k in range(D):
                    ostg = stg_pool.tile([P, MHALF], F32, tag="STG",
                                         name=f"ostg{h}{k}")
                    nc.scalar.activation(ostg[:], lanes(O[h][:], MHALF)[k],
                                         Copy, scale=c0_ap)
                    nc.sync.dma_start(
                        out_d[P * k:P * (k + 1), MHALF * h:MHALF * (h + 1)],
                        ostg[:])

            g1(0)
            scans(0)
            g1(1)
            scans(1)
            g2(0)
            mix(0)
            g2(1)
            mix(1)
            g3(0)
            out_half(0)
            g3(1)
            out_half(1)

    nc.compile()
    _PROGRAM_CACHE[key] = nc
    return nc


def _run(inputs, trace=False, trace_kwargs=None):
    flux = np.ascontiguousarray(np.asarray(inputs["flux"], dtype=np.float32))
    eta = float(np.asarray(inputs["eta"]).reshape(-1)[0])
    tile_idx = np.asarray(inputs["tile_idx"]).astype(np.int64).reshape(-1)
    fib_idx = np.asarray(inputs["fib_idx"]).astype(np.int64).reshape(-1)

    assert flux.shape == (N_WAVE, N_POS), flux.shape
    assert tile_idx.shape == (N_POS,) and fib_idx.shape == (N_POS,)

    halves = _host_precompute(tile_idx, fib_idx)
    c0 = np.float32(1.0 - 2.0 * eta)
    etap = np.float32(eta / float(c0))
    consts = np.tile(np.array([[c0, etap]], np.float32), (P, 1))

    nc = _build_program([halves[0]["n_p"], halves[1]["n_p"]])

    shared = {"consts": consts}
    for h in range(2):
        shared[f"sort_idx{h}"] = halves[h]["sort_idx"]
        shared[f"nat_idx{h}"] = halves[h]["nat_idx"]
        shared[f"out_idx{h}"] = halves[h]["out_idx"]
        shared[f"cmask{h}"] = halves[h]["cmask"]

    in_maps = []
    for c in range(N_CORES):
        m = dict(shared)
        m["flux"] = np.ascontiguousarray(
            flux[c * ROWS_PER_CORE:(c + 1) * ROWS_PER_CORE])
        in_maps.append(m)

    kwargs = {}
    if trace:
        kwargs["trace"] = True
        if trace_kwargs:
            kwargs["trace_kwargs"] = trace_kwargs
    res = run_bass_kernel_spmd(nc, in_maps, core_ids=list(range(N_CORES)),
                               **kwargs)
    out = np.concatenate([r["out"] for r in res.results], axis=0)
    return out, res


def kernel(**inputs) -> np.ndarray:
    out, _ = _run(inputs, trace=False)
    return out


# revision 3
# speedup vs baseline: 3.1317x; 1.0064x over previous
"""Trainium2 Bass kernel for nn_CrossTalk (segment scatter-add -> tridiag mix -> gather).

Full (unsharded) inputs in, full output out. Shards the wavelength axis of
flux across 8 NeuronCores (512 rows each). Per core, the 512 rows are packed
as FOUR bf16 lanes interleaved along the free axis ([128, 7200, 4]), so each
gpsimd ap_gather index moves all four row-chunks at once (gather cost is
per-index, not per-byte: ~24-29ns/idx at d=4, same as d=1).

Positions are processed in two halves split at the tile-6 boundary of the
sorted order, which (a) fits SBUF and (b) pipelines DVE scan/mix work under
the gpsimd gathers. Late stages are further sub-split so conversions/DMA
overlap the last gathers:

  G1h   gpsimd.ap_gather : sort flux columns of half h by segment id
  DVE   tensor_tensor_scan x4 lanes: segmented prefix-sum (strided views)
  G2h*  gpsimd.ap_gather : run-end sums -> per-tile padded natural layout
  DVE   tt + stt         : tridiag mix M = C + (eta/c0) * (L + R)
  G3h*  gpsimd.ap_gather : gather M back to original positions
  ACT/DVE copy scale=c0  : bf16 -> f32, final scale folded in
  DMA   out

Precision: values ride in bf16 (scan state is fp32 internally); measured
rel-err vs the f32 reference ~6e-3 (harness gate 2e-2).
"""

import os
import sys

import numpy as np

for _p in ("/opt/trn_rl_repo", "/root/.axon_site/_ro/trn_rl_repo"):
    if os.path.isdir(_p) and _p not in sys.path:
        sys.path.insert(0, _p)

import ml_dtypes  # noqa: E402

import concourse.bacc as bacc  # noqa: E402
import concourse.mybir as mybir  # noqa: E402
from concourse.tile import TileContext  # noqa: E402
from concourse.bass_utils import run_bass_kernel_spmd  # noqa: E402

# Problem geometry (fixed by the harness spec).
N_WAVE = 4096
N_TILES = 12
N_FIBRES = 600
N_POS = N_TILES * N_FIBRES          # 7200
N_CORES = 8
ROWS_PER_CORE = N_WAVE // N_CORES   # 512
P = 128                             # SBUF partitions
D = 4                               # row-chunks packed per gather payload
PAD_W = N_FIBRES + 4                # 604 per-tile padded width
HALF_TILES = N_TILES // 2           # 6
NAT_H = HALF_TILES * PAD_W          # 3624 natural slots per half
MHALF = HALF_TILES * N_FIBRES       # 3600 compact M columns per half

# sub-splits of the late stages (tile counts / output column ranges)
G2_SPLITS = {0: [(0, 6)], 1: [(0, 3), (3, 3)]}          # (tile_lo, n_tiles)
G3_SPLITS = {0: [(0, 3600)], 1: [(3600, 1800), (5400, 1800)]}  # (col, n)

F32 = mybir.dt.float32
BF16 = mybir.dt.bfloat16
I16 = mybir.dt.int16

_PROGRAM_CACHE = {}


def _ceil(x, m):
    return (x + m - 1) // m * m


def _wrap_idx(flat):
    """ap_gather idx layout: flat list wrapped over 16 partitions, padded so
    the ucode's ceil_div(num_idxs,32) uint32 reads stay in bounds; tiled to
    128 partitions."""
    flat = np.asarray(flat, np.int64)
    padded = _ceil(flat.size, 32)
    if padded != flat.size:
        flat = np.concatenate([flat, np.zeros(padded - flat.size, np.int64)])
    w = flat.reshape(padded // 16, 16).T.astype(np.int16)   # [16, S]
    return np.tile(w, (P // 16, 1))                          # [128, S]


def _host_precompute(tile_idx, fib_idx):
    seg = (tile_idx.astype(np.int64) * N_FIBRES + fib_idx.astype(np.int64))
    order = np.argsort(seg, kind="stable")
    sseg = seg[order]

    is_end = np.ones(N_POS, bool)
    is_end[:-1] = sseg[1:] != sseg[:-1]
    end_j = np.nonzero(is_end)[0]

    o6 = int(np.searchsorted(sseg, HALF_TILES * N_FIBRES))  # sorted split
    halves = []
    for h in range(2):
        lo, hi = (0, o6) if h == 0 else (o6, N_POS)
        n = hi - lo
        n_p = _ceil(n, 16)

        sort_flat = np.zeros(n_p, np.int64)
        sort_flat[:n] = order[lo:hi]

        cmask = np.zeros(n_p, np.float32)
        cmask[1:n] = (sseg[lo + 1:hi] == sseg[lo:hi - 1]).astype(np.float32)

        # bin -> local run-end column in S_h (leading zero block at 0)
        endcol = np.zeros(N_TILES * N_FIBRES, np.int64)
        sel = (end_j >= lo) & (end_j < hi)
        ej = end_j[sel]
        endcol[sseg[ej]] = ej - lo + 1

        nat = np.zeros(NAT_H, np.int64)
        nat.reshape(HALF_TILES, PAD_W)[:, 1:N_FIBRES + 1] = (
            endcol.reshape(N_TILES, N_FIBRES)[h * HALF_TILES:(h + 1) * HALF_TILES]
        )

        halves.append({
            "n": n,
            "n_p": n_p,
            "sort_idx": _wrap_idx(sort_flat),
            "nat_idx": [_wrap_idx(nat[t0 * PAD_W:(t0 + nt) * PAD_W])
                        for t0, nt in G2_SPLITS[h]],
            "out_idx": [_wrap_idx(seg[c:c + nc_]) for c, nc_ in G3_SPLITS[h]],
            "cmask": np.tile(
                cmask.astype(ml_dtypes.bfloat16)[None, :], (P, 1)),
        })
    return halves


def _build_program(n_p):
    key = ("v3", tuple(n_p))
    if key in _PROGRAM_CACHE:
        return _PROGRAM_CACHE[key]

    nc = bacc.Bacc("TRN2", target_bir_lowering=False, debug=False)

    flux_d = nc.dram_tensor("flux", [ROWS_PER_CORE, N_POS], F32,
                            kind="ExternalInput").ap()
    out_d = nc.dram_tensor("out", [ROWS_PER_CORE, N_POS], F32,
                           kind="ExternalOutput").ap()
    consts_d = nc.dram_tensor("consts", [P, 2], F32, kind="ExternalInput").ap()
    sort_d, nat_d, oidx_d, cmask_d = [], [], [], []
    for h in range(2):
        sort_d.append(nc.dram_tensor(
            f"sort_idx{h}", [P, _ceil(n_p[h], 32) // 16], I16,
            kind="ExternalInput").ap())
        nat_d.append([nc.dram_tensor(
            f"nat_idx{h}_{i}", [P, _ceil(nt * PAD_W, 32) // 16], I16,
            kind="ExternalInput").ap()
            for i, (t0, nt) in enumerate(G2_SPLITS[h])])
        oidx_d.append([nc.dram_tensor(
            f"out_idx{h}_{i}", [P, _ceil(nc_, 32) // 16], I16,
            kind="ExternalInput").ap()
            for i, (c, nc_) in enumerate(G3_SPLITS[h])])
        cmask_d.append(nc.dram_tensor(
            f"cmask{h}", [P, n_p[h]], BF16, kind="ExternalInput").ap())

    mult = mybir.AluOpType.mult
    add = mybir.AluOpType.add
    Copy = mybir.ActivationFunctionType.Copy

    def lanes(ap_flat, nblk):
        """[P, nblk*D] flat bf16 -> list of D strided [P, nblk] views."""
        v = ap_flat.rearrange("p (j k) -> p j k", k=D)
        return [v[:, :, k:k + 1].squeeze(2) for k in range(D)]

    with TileContext(nc) as tc:
        with (
            tc.tile_pool(name="pers", bufs=1) as pers,
            tc.tile_pool(name="big", bufs=1) as big_pool,
            tc.tile_pool(name="half", bufs=3) as half_pool,
            tc.tile_pool(name="stg", bufs=2) as stg_pool,
        ):
            sort_t, nat_t, oidx_t, cmask_t = [], [], [], []
            for h in range(2):
                st = pers.tile([P, _ceil(n_p[h], 32) // 16], I16, tag=f"si{h}",
                               name=f"sort{h}")
                nc.sync.dma_start(st[:], sort_d[h])
                nts = []
                for i, (t0, nt_) in enumerate(G2_SPLITS[h]):
                    t = pers.tile([P, _ceil(nt_ * PAD_W, 32) // 16], I16,
                                  tag=f"ni{h}_{i}", name=f"nat{h}_{i}")
                    nc.sync.dma_start(t[:], nat_d[h][i])
                    nts.append(t)
                ots = []
                for i, (c, nc_) in enumerate(G3_SPLITS[h]):
                    t = pers.tile([P, _ceil(nc_, 32) // 16], I16,
                                  tag=f"oi{h}_{i}", name=f"oidx{h}_{i}")
                    nc.sync.dma_start(t[:], oidx_d[h][i])
                    ots.append(t)
                mt = pers.tile([P, n_p[h]], BF16, tag=f"cm{h}", name=f"cmask{h}")
                nc.sync.dma_start(mt[:], cmask_d[h])
                sort_t.append(st)
                nat_t.append(nts)
                oidx_t.append(ots)
                cmask_t.append(mt)
            consts_t = pers.tile([P, 2], F32, tag="consts")
            nc.sync.dma_start(consts_t[:], consts_d)
            c0_ap = consts_t[:, 0:1]     # 1 - 2*eta (final output scale)
            etap_ap = consts_t[:, 1:2]   # eta / (1 - 2*eta)

            # S tiles upfront so their zero blocks don't serialize later
            # engine streams.
            S = [half_pool.tile([P, (1 + n_p[h]) * D], BF16, tag="HALF",
                                name=f"S{h}") for h in range(2)]
            nc.vector.memset(S[0][:, 0:D], 0.0)
            nc.vector.memset(S[1][:, 0:D], 0.0)

            # ---- load + pack flux into F [P, N_POS, D] bf16 ----
            F = big_pool.tile([P, N_POS * D], BF16, tag="BIG", name="F")
            for k in range(D):
                for h in range(2):
                    fin = stg_pool.tile([P, MHALF], F32, tag="STG",
                                        name=f"fin{k}{h}")
                    nc.sync.dma_start(
                        fin[:], flux_d[P * k:P * (k + 1),
                                       MHALF * h:MHALF * (h + 1)])
                    dst = lanes(F[:, MHALF * h * D:MHALF * (h + 1) * D],
                                MHALF)[k]
                    if k < 2:
                        nc.scalar.activation(dst, fin[:], Copy)
                    else:
                        nc.vector.tensor_scalar(dst, fin[:], 0.0, None, add)

            N = [None, None]
            M = big_pool.tile([P, N_POS * D], BF16, tag="BIG", name="M")
            O = [None, None]

            def g1(h):
                nc.gpsimd.ap_gather(S[h][:, D:], F[:], sort_t[h][:],
                                    channels=P, num_elems=N_POS, d=D,
                                    num_idxs=n_p[h])

            def scans(h):
                for lv in lanes(S[h][:, D:], n_p[h]):
                    nc.vector.tensor_tensor_scan(
                        lv, cmask_t[h][:], lv, 0.0, mult, add)

            def g2(h, i):
                t0, nt_ = G2_SPLITS[h][i]
                nc.gpsimd.ap_gather(
                    N[h][:, t0 * PAD_W * D:(t0 + nt_) * PAD_W * D],
                    S[h][:], nat_t[h][i][:],
                    channels=P, num_elems=1 + n_p[h], d=D,
                    num_idxs=nt_ * PAD_W)

            def mix(h, i):
                t0, nt_ = G2_SPLITS[h][i]
                nv = N[h][:, t0 * PAD_W * D:(t0 + nt_) * PAD_W * D].rearrange(
                    "p (t w k) -> p t w k", w=PAD_W, k=D)
                g0 = (h * HALF_TILES + t0) * N_FIBRES
                mv = M[:, g0 * D:(g0 + nt_ * N_FIBRES) * D].rearrange(
                    "p (t w k) -> p t w k", w=N_FIBRES, k=D)
                nc.vector.tensor_tensor(
                    mv, nv[:, :, 0:N_FIBRES, :],
                    nv[:, :, 2:N_FIBRES + 2, :], add)
                nc.vector.scalar_tensor_tensor(
                    mv, mv, etap_ap, nv[:, :, 1:N_FIBRES + 1, :], mult, add)

            def g3(h, i):
                c, nc_ = G3_SPLITS[h][i]
                off = (c - 3600 * h) * D
                nc.gpsimd.ap_gather(O[h][:, off:off + nc_ * D], M[:],
                                    oidx_t[h][i][:],
                                    channels=P, num_elems=N_POS, d=D,
                                    num_idxs=nc_)

            def out_cols(h, i):
                c, nc_ = G3_SPLITS[h][i]
                off = (c - 3600 * h) * D
                ov = O[h][:, off:off + nc_ * D]
                for k in range(D):
                    ostg = stg_pool.tile([P, nc_], F32, tag="STG",
                                         name=f"ostg{h}{i}{k}")
                    lane = lanes(ov, nc_)[k]
                    if k < 2:
                        nc.scalar.activation(ostg[:], lane, Copy, scale=c0_ap)
                    else:
                        nc.vector.tensor_scalar(ostg[:], lane, c0_ap, None,
                                                mult)
                    nc.sync.dma_start(
                        out_d[P * k:P * (k + 1), c:c + nc_], ostg[:])

            g1(0)
            scans(0)
            g1(1)
            scans(1)
            N[0] = half_pool.tile([P, NAT_H * D], BF16, tag="HALF", name="N0")
            g2(0, 0)
            mix(0, 0)
            N[1] = half_pool.tile([P, NAT_H * D], BF16, tag="HALF", name="N1")
            g2(1, 0)
            mix(1, 0)
            g2(1, 1)
            mix(1, 1)
            O[0] = half_pool.tile([P, MHALF * D], BF16, tag="HALF", name="O0")
            g3(0, 0)
            out_cols(0, 0)
            O[1] = half_pool.tile([P, MHALF * D], BF16, tag="HALF", name="O1")
            g3(1, 0)
            out_cols(1, 0)
            g3(1, 1)
            out_cols(1, 1)

    nc.compile()
    _PROGRAM_CACHE[key] = nc
    return nc


def _run(inputs, trace=False, trace_kwargs=None):
    flux = np.ascontiguousarray(np.asarray(inputs["flux"], dtype=np.float32))
    eta = float(np.asarray(inputs["eta"]).reshape(-1)[0])
    tile_idx = np.asarray(inputs["tile_idx"]).astype(np.int64).reshape(-1)
    fib_idx = np.asarray(inputs["fib_idx"]).astype(np.int64).reshape(-1)

    assert flux.shape == (N_WAVE, N_POS), flux.shape
    assert tile_idx.shape == (N_POS,) and fib_idx.shape == (N_POS,)

    halves = _host_precompute(tile_idx, fib_idx)
    c0 = np.float32(1.0 - 2.0 * eta)
    etap = np.float32(eta / float(c0))
    consts = np.tile(np.array([[c0, etap]], np.float32), (P, 1))

    nc = _build_program([halves[0]["n_p"], halves[1]["n_p"]])

    shared = {"consts": consts}
    for h in range(2):
        shared[f"sort_idx{h}"] = halves[h]["sort_idx"]
        shared[f"cmask{h}"] = halves[h]["cmask"]
        for i in range(len(G2_SPLITS[h])):
            shared[f"nat_idx{h}_{i}"] = halves[h]["nat_idx"][i]
        for i in range(len(G3_SPLITS[h])):
            shared[f"out_idx{h}_{i}"] = halves[h]["out_idx"][i]

    in_maps = []
    for c in range(N_CORES):
        m = dict(shared)
        m["flux"] = np.ascontiguousarray(
            flux[c * ROWS_PER_CORE:(c + 1) * ROWS_PER_CORE])
        in_maps.append(m)

    kwargs = {}
    if trace:
        kwargs["trace"] = True
        if trace_kwargs:
            kwargs["trace_kwargs"] = trace_kwargs
    res = run_bass_kernel_spmd(nc, in_maps, core_ids=list(range(N_CORES)),
                               **kwargs)
    out = np.concatenate([r["out"] for r in res.results], axis=0)
    return out, res


def kernel(**inputs) -> np.ndarray:
    out, _ = _run(inputs, trace=False)
    return out


# revision 6
# speedup vs baseline: 3.1841x; 1.0167x over previous
"""Trainium2 Bass kernel for nn_CrossTalk (segment scatter-add -> tridiag mix -> gather).

Full (unsharded) inputs in, full output out. Shards the wavelength axis of
flux across 8 NeuronCores (512 rows each). Per core, the 512 rows are packed
as FOUR bf16 lanes interleaved along the free axis ([128, 7200, 4]), so each
gpsimd ap_gather index moves all four row-chunks at once (gather cost is
per-index, not per-byte: ~24-29ns/idx at d=4, same as d=1).

Positions are processed in two halves split at the tile-6 boundary of the
sorted order, which (a) fits SBUF and (b) pipelines DVE scan/mix work under
the gpsimd gathers. Late stages are further sub-split so conversions/DMA
overlap the last gathers:

  G1h   gpsimd.ap_gather : sort flux columns of half h by segment id
  DVE   tensor_tensor_scan x4 lanes: segmented prefix-sum (strided views)
  G2h*  gpsimd.ap_gather : run-end sums -> per-tile padded natural layout
  DVE   tt + stt         : tridiag mix M = C + (eta/c0) * (L + R)
  G3h*  gpsimd.ap_gather : gather M back to original positions
  ACT/DVE copy scale=c0  : bf16 -> f32, final scale folded in
  DMA   out

Precision: values ride in bf16 (scan state is fp32 internally); measured
rel-err vs the f32 reference ~6e-3 (harness gate 2e-2).
"""

import os
import sys

import numpy as np

for _p in ("/opt/trn_rl_repo", "/root/.axon_site/_ro/trn_rl_repo"):
    if os.path.isdir(_p) and _p not in sys.path:
        sys.path.insert(0, _p)

import ml_dtypes  # noqa: E402

import concourse.bacc as bacc  # noqa: E402
import concourse.mybir as mybir  # noqa: E402
from concourse.tile import TileContext  # noqa: E402
from concourse.bass_utils import run_bass_kernel_spmd  # noqa: E402

# Problem geometry (fixed by the harness spec).
N_WAVE = 4096
N_TILES = 12
N_FIBRES = 600
N_POS = N_TILES * N_FIBRES          # 7200
N_CORES = 8
ROWS_PER_CORE = N_WAVE // N_CORES   # 512
P = 128                             # SBUF partitions
D = 4                               # row-chunks packed per gather payload
PAD_W = N_FIBRES + 4                # 604 per-tile padded width
HALF_TILES = N_TILES // 2           # 6
NAT_H = HALF_TILES * PAD_W          # 3624 natural slots per half
MHALF = HALF_TILES * N_FIBRES       # 3600 compact M columns per half

# sub-splits of the late stages (tile counts / output column ranges)
G2_SPLITS = {0: [(0, 6)], 1: [(0, 3), (3, 3)]}          # (tile_lo, n_tiles)
G3_SPLITS = {0: [(0, 3600)], 1: [(3600, 1800), (5400, 1800)]}  # (col, n)

F32 = mybir.dt.float32
BF16 = mybir.dt.bfloat16
I16 = mybir.dt.int16

_PROGRAM_CACHE = {}


def _ceil(x, m):
    return (x + m - 1) // m * m


def _wrap_idx(flat):
    """ap_gather idx layout: flat list wrapped over 16 partitions, padded so
    the ucode's ceil_div(num_idxs,32) uint32 reads stay in bounds; tiled to
    128 partitions."""
    flat = np.asarray(flat, np.int64)
    padded = _ceil(flat.size, 32)
    if padded != flat.size:
        flat = np.concatenate([flat, np.zeros(padded - flat.size, np.int64)])
    w = flat.reshape(padded // 16, 16).T.astype(np.int16)   # [16, S]
    return np.tile(w, (P // 16, 1))                          # [128, S]


def _host_precompute(tile_idx, fib_idx):
    seg = (tile_idx.astype(np.int64) * N_FIBRES + fib_idx.astype(np.int64))
    order = np.argsort(seg, kind="stable")
    sseg = seg[order]

    is_end = np.ones(N_POS, bool)
    is_end[:-1] = sseg[1:] != sseg[:-1]
    end_j = np.nonzero(is_end)[0]

    o6 = int(np.searchsorted(sseg, HALF_TILES * N_FIBRES))  # sorted split
    halves = []
    for h in range(2):
        lo, hi = (0, o6) if h == 0 else (o6, N_POS)
        n = hi - lo
        n_p = _ceil(n, 16)

        sort_flat = np.zeros(n_p, np.int64)
        sort_flat[:n] = order[lo:hi]

        cmask = np.zeros(n_p, np.float32)
        cmask[1:n] = (sseg[lo + 1:hi] == sseg[lo:hi - 1]).astype(np.float32)

        # bin -> local run-end column in S_h (leading zero block at 0)
        endcol = np.zeros(N_TILES * N_FIBRES, np.int64)
        sel = (end_j >= lo) & (end_j < hi)
        ej = end_j[sel]
        endcol[sseg[ej]] = ej - lo + 1

        nat = np.zeros(NAT_H, np.int64)
        nat.reshape(HALF_TILES, PAD_W)[:, 1:N_FIBRES + 1] = (
            endcol.reshape(N_TILES, N_FIBRES)[h * HALF_TILES:(h + 1) * HALF_TILES]
        )

        halves.append({
            "n": n,
            "n_p": n_p,
            "sort_idx": _wrap_idx(sort_flat),
            "nat_idx": [_wrap_idx(nat[t0 * PAD_W:(t0 + nt) * PAD_W])
                        for t0, nt in G2_SPLITS[h]],
            "out_idx": [_wrap_idx(seg[c:c + nc_]) for c, nc_ in G3_SPLITS[h]],
            "cmask": np.tile(
                cmask.astype(ml_dtypes.bfloat16)[None, :], (P, 1)),
        })
    return halves


def _build_program(n_p):
    key = ("v3", tuple(n_p))
    if key in _PROGRAM_CACHE:
        return _PROGRAM_CACHE[key]

    nc = bacc.Bacc("TRN2", target_bir_lowering=False, debug=False)

    flux_d = nc.dram_tensor("flux", [ROWS_PER_CORE, N_POS], F32,
                            kind="ExternalInput").ap()
    out_d = nc.dram_tensor("out", [ROWS_PER_CORE, N_POS], F32,
                           kind="ExternalOutput").ap()
    consts_d = nc.dram_tensor("consts", [P, 2], F32, kind="ExternalInput").ap()
    sort_d, nat_d, oidx_d, cmask_d = [], [], [], []
    for h in range(2):
        sort_d.append(nc.dram_tensor(
            f"sort_idx{h}", [P, _ceil(n_p[h], 32) // 16], I16,
            kind="ExternalInput").ap())
        nat_d.append([nc.dram_tensor(
            f"nat_idx{h}_{i}", [P, _ceil(nt * PAD_W, 32) // 16], I16,
            kind="ExternalInput").ap()
            for i, (t0, nt) in enumerate(G2_SPLITS[h])])
        oidx_d.append([nc.dram_tensor(
            f"out_idx{h}_{i}", [P, _ceil(nc_, 32) // 16], I16,
            kind="ExternalInput").ap()
            for i, (c, nc_) in enumerate(G3_SPLITS[h])])
        cmask_d.append(nc.dram_tensor(
            f"cmask{h}", [P, n_p[h]], BF16, kind="ExternalInput").ap())

    mult = mybir.AluOpType.mult
    add = mybir.AluOpType.add
    Copy = mybir.ActivationFunctionType.Copy

    def lanes(ap_flat, nblk):
        """[P, nblk*D] flat bf16 -> list of D strided [P, nblk] views."""
        v = ap_flat.rearrange("p (j k) -> p j k", k=D)
        return [v[:, :, k:k + 1].squeeze(2) for k in range(D)]

    with TileContext(nc) as tc:
        with (
            tc.tile_pool(name="pers", bufs=1) as pers,
            tc.tile_pool(name="big", bufs=1) as big_pool,
            tc.tile_pool(name="half", bufs=3) as half_pool,
            tc.tile_pool(name="stg", bufs=3) as stg_pool,
        ):
            sort_t, nat_t, oidx_t, cmask_t = [], [], [], []
            for h in range(2):
                st = pers.tile([P, _ceil(n_p[h], 32) // 16], I16, tag=f"si{h}",
                               name=f"sort{h}")
                nc.sync.dma_start(st[:], sort_d[h])
                nts = []
                for i, (t0, nt_) in enumerate(G2_SPLITS[h]):
                    t = pers.tile([P, _ceil(nt_ * PAD_W, 32) // 16], I16,
                                  tag=f"ni{h}_{i}", name=f"nat{h}_{i}")
                    nc.sync.dma_start(t[:], nat_d[h][i])
                    nts.append(t)
                ots = []
                for i, (c, nc_) in enumerate(G3_SPLITS[h]):
                    t = pers.tile([P, _ceil(nc_, 32) // 16], I16,
                                  tag=f"oi{h}_{i}", name=f"oidx{h}_{i}")
                    nc.sync.dma_start(t[:], oidx_d[h][i])
                    ots.append(t)
                mt = pers.tile([P, n_p[h]], BF16, tag=f"cm{h}", name=f"cmask{h}")
                nc.sync.dma_start(mt[:], cmask_d[h])
                sort_t.append(st)
                nat_t.append(nts)
                oidx_t.append(ots)
                cmask_t.append(mt)
            consts_t = pers.tile([P, 2], F32, tag="consts")
            nc.sync.dma_start(consts_t[:], consts_d)
            c0_ap = consts_t[:, 0:1]     # 1 - 2*eta (final output scale)
            etap_ap = consts_t[:, 1:2]   # eta / (1 - 2*eta)

            # S tiles upfront so their zero blocks don't serialize later
            # engine streams.
            S = [half_pool.tile([P, (1 + n_p[h]) * D], BF16, tag="HALF",
                                name=f"S{h}") for h in range(2)]
            nc.vector.memset(S[0][:, 0:D], 0.0)
            nc.vector.memset(S[1][:, 0:D], 0.0)

            # ---- load + pack flux into F [P, N_POS, D] bf16 ----
            F = big_pool.tile([P, N_POS * D], BF16, tag="BIG", name="F")
            for k in range(D):
                for h in range(2):
                    fin = stg_pool.tile([P, MHALF], F32, tag="STG",
                                        name=f"fin{k}{h}")
                    nc.sync.dma_start(
                        fin[:], flux_d[P * k:P * (k + 1),
                                       MHALF * h:MHALF * (h + 1)])
                    dst = lanes(F[:, MHALF * h * D:MHALF * (h + 1) * D],
                                MHALF)[k]
                    if k < 1:
                        nc.scalar.activation(dst, fin[:], Copy)
                    else:
                        nc.vector.tensor_scalar(dst, fin[:], 0.0, None, add)

            N = [None, None]
            M = big_pool.tile([P, N_POS * D], BF16, tag="BIG", name="M")
            O = [None, None]

            def g1(h):
                nc.gpsimd.ap_gather(S[h][:, D:], F[:], sort_t[h][:],
                                    channels=P, num_elems=N_POS, d=D,
                                    num_idxs=n_p[h])

            def scans(h):
                for lv in lanes(S[h][:, D:], n_p[h]):
                    nc.vector.tensor_tensor_scan(
                        lv, cmask_t[h][:], lv, 0.0, mult, add)

            def g2(h, i):
                t0, nt_ = G2_SPLITS[h][i]
                nc.gpsimd.ap_gather(
                    N[h][:, t0 * PAD_W * D:(t0 + nt_) * PAD_W * D],
                    S[h][:], nat_t[h][i][:],
                    channels=P, num_elems=1 + n_p[h], d=D,
                    num_idxs=nt_ * PAD_W)

            def mix(h, i):
                t0, nt_ = G2_SPLITS[h][i]
                nv = N[h][:, t0 * PAD_W * D:(t0 + nt_) * PAD_W * D].rearrange(
                    "p (t w k) -> p t w k", w=PAD_W, k=D)
                g0 = (h * HALF_TILES + t0) * N_FIBRES
                mv = M[:, g0 * D:(g0 + nt_ * N_FIBRES) * D].rearrange(
                    "p (t w k) -> p t w k", w=N_FIBRES, k=D)
                nc.vector.tensor_tensor(
                    mv, nv[:, :, 0:N_FIBRES, :],
                    nv[:, :, 2:N_FIBRES + 2, :], add)
                nc.vector.scalar_tensor_tensor(
                    mv, mv, etap_ap, nv[:, :, 1:N_FIBRES + 1, :], mult, add)

            def g3(h, i):
                c, nc_ = G3_SPLITS[h][i]
                off = (c - 3600 * h) * D
                nc.gpsimd.ap_gather(O[h][:, off:off + nc_ * D], M[:],
                                    oidx_t[h][i][:],
                                    channels=P, num_elems=N_POS, d=D,
                                    num_idxs=nc_)

            def out_cols(h, i):
                c, nc_ = G3_SPLITS[h][i]
                off = (c - 3600 * h) * D
                ov = O[h][:, off:off + nc_ * D]
                for k in range(D):
                    ostg = stg_pool.tile([P, nc_], F32, tag="STG",
                                         name=f"ostg{h}{i}{k}")
                    lane = lanes(ov, nc_)[k]
                    if k < 2:
                        nc.scalar.activation(ostg[:], lane, Copy, scale=c0_ap)
                    else:
                        nc.vector.tensor_scalar(ostg[:], lane, c0_ap, None,
                                                mult)
                    nc.sync.dma_start(
                        out_d[P * k:P * (k + 1), c:c + nc_], ostg[:])

            # Emission order doubles as per-engine issue order; gpsimd
            # gathers are emitted before the DVE work of the NEXT stage so
            # no gather ends up ordered behind DVE ops it doesn't need.
            g1(0)
            g1(1)
            scans(0)
            N[0] = half_pool.tile([P, NAT_H * D], BF16, tag="HALF", name="N0")
            g2(0, 0)
            scans(1)
            N[1] = half_pool.tile([P, NAT_H * D], BF16, tag="HALF", name="N1")
            g2(1, 0)
            g2(1, 1)
            mix(0, 0)
            mix(1, 0)
            mix(1, 1)
            O[0] = half_pool.tile([P, MHALF * D], BF16, tag="HALF", name="O0")
            g3(0, 0)
            O[1] = half_pool.tile([P, MHALF * D], BF16, tag="HALF", name="O1")
            g3(1, 0)
            g3(1, 1)
            out_cols(0, 0)
            out_cols(1, 0)
            out_cols(1, 1)

    nc.compile()
    _PROGRAM_CACHE[key] = nc
    return nc


def _run(inputs, trace=False, trace_kwargs=None):
    flux = np.ascontiguousarray(np.asarray(inputs["flux"], dtype=np.float32))
    eta = float(np.asarray(inputs["eta"]).reshape(-1)[0])
    tile_idx = np.asarray(inputs["tile_idx"]).astype(np.int64).reshape(-1)
    fib_idx = np.asarray(inputs["fib_idx"]).astype(np.int64).reshape(-1)

    assert flux.shape == (N_WAVE, N_POS), flux.shape
    assert tile_idx.shape == (N_POS,) and fib_idx.shape == (N_POS,)

    halves = _host_precompute(tile_idx, fib_idx)
    c0 = np.float32(1.0 - 2.0 * eta)
    etap = np.float32(eta / float(c0))
    consts = np.tile(np.array([[c0, etap]], np.float32), (P, 1))

    nc = _build_program([halves[0]["n_p"], halves[1]["n_p"]])

    shared = {"consts": consts}
    for h in range(2):
        shared[f"sort_idx{h}"] = halves[h]["sort_idx"]
        shared[f"cmask{h}"] = halves[h]["cmask"]
        for i in range(len(G2_SPLITS[h])):
            shared[f"nat_idx{h}_{i}"] = halves[h]["nat_idx"][i]
        for i in range(len(G3_SPLITS[h])):
            shared[f"out_idx{h}_{i}"] = halves[h]["out_idx"][i]

    in_maps = []
    for c in range(N_CORES):
        m = dict(shared)
        m["flux"] = np.ascontiguousarray(
            flux[c * ROWS_PER_CORE:(c + 1) * ROWS_PER_CORE])
        in_maps.append(m)

    kwargs = {}
    if trace:
        kwargs["trace"] = True
        if trace_kwargs:
            kwargs["trace_kwargs"] = trace_kwargs
    res = run_bass_kernel_spmd(nc, in_maps, core_ids=list(range(N_CORES)),
                               **kwargs)
    out = np.concatenate([r["out"] for r in res.results], axis=0)
    return out, res


def kernel(**inputs) -> np.ndarray:
    out, _ = _run(inputs, trace=False)
    return out


# revision 7
# speedup vs baseline: 4.5632x; 1.4331x over previous
"""Trainium2 Bass kernel for nn_CrossTalk, v6: gathers + tensor-engine middle.

Full (unsharded) inputs in, full output out; wavelength axis sharded over 8
NeuronCores (512 rows/core). Per core the 512 rows ride as FOUR bf16 lanes
interleaved along the free axis ([128, 7200, 4]) so each gpsimd ap_gather
index moves all four row-chunks at once (~24-30ns/idx regardless of d).

Pipeline per core:
  G1h  gpsimd.ap_gather       : sort flux columns by segment id (2 halves,
                                split at the tile-6 boundary)
  PE   transpose + matmul     : per 128-bin window, per lane: accumulate
                                psum[wl, bins] = sum_blocks fluxT_ck @ W_cw
                                where W encodes BOTH the segment-sum one-hot
                                AND the tridiagonal crosstalk band (1 on the
                                bin, eta/c0 on same-tile neighbours) in f32
  DVE  tensor_scalar (1-input): psum -> M bf16 interleaved (strided copy)
  G3q  gpsimd.ap_gather       : gather M back to original positions, 4
                                quarter-gathers overlapped with output
                                conversion + DMA

The middle stage runs entirely on PE + Scalar + DVE-1-input ops, all of
which are measured contention-free against gpsimd gathers (unlike 2-input
DVE ops, which stall them ~1:1). Scan/segmented-sum DVE work is gone: the
matmul does the binning in f32 PSUM.

Precision: bf16 data path with f32 accumulation; rel err vs f32 reference
~6e-3 (harness gate 2e-2).
"""

import os
import sys

import numpy as np

for _p in ("/opt/trn_rl_repo", "/root/.axon_site/_ro/trn_rl_repo"):
    if os.path.isdir(_p) and _p not in sys.path:
        sys.path.insert(0, _p)

import ml_dtypes  # noqa: E402

import concourse.bacc as bacc  # noqa: E402
import concourse.mybir as mybir  # noqa: E402
from concourse.bass import MemorySpace  # noqa: E402
from concourse.tile import TileContext  # noqa: E402
from concourse.bass_utils import run_bass_kernel_spmd  # noqa: E402

N_WAVE = 4096
N_TILES = 12
N_FIBRES = 600
N_POS = N_TILES * N_FIBRES          # 7200
N_CORES = 8
ROWS_PER_CORE = N_WAVE // N_CORES   # 512
P = 128
D = 4                               # row-chunks per gather payload
HALF_TILES = N_TILES // 2
MHALF = HALF_TILES * N_FIBRES       # 3600 bins per half
WIN = 128                           # bin-window width for the PE stage
QUARTER = N_POS // 4                # 1800 output cols per G3 sub-gather
INQ = N_POS // 4                    # input load chunk (cols)

F32 = mybir.dt.float32
BF16 = mybir.dt.bfloat16
I16 = mybir.dt.int16

_PROGRAM_CACHE = {}


def _ceil(x, m):
    return (x + m - 1) // m * m


def _wrap_idx(flat):
    flat = np.asarray(flat, np.int64)
    padded = _ceil(flat.size, 32)
    if padded != flat.size:
        flat = np.concatenate([flat, np.zeros(padded - flat.size, np.int64)])
    w = flat.reshape(padded // 16, 16).T.astype(np.int16)
    return np.tile(w, (P // 16, 1))


def _host_precompute(tile_idx, fib_idx, etap):
    seg = tile_idx.astype(np.int64) * N_FIBRES + fib_idx.astype(np.int64)
    order = np.argsort(seg, kind="stable")
    sseg = seg[order]
    o6 = int(np.searchsorted(sseg, MHALF))

    halves = []
    for h in range(2):
        lo, hi = (0, o6) if h == 0 else (o6, N_POS)
        n = hi - lo
        n_p = _ceil(n, 16)      # gather count (mult of 4; 16 for wrap ease)
        n_pb = _ceil(n_p, 128)  # S width in blocks of 128
        nblk = n_pb // 128

        sort_flat = np.zeros(n_p, np.int64)
        sort_flat[:n] = order[lo:hi]

        # windows of WIN bins; per window the contributing sorted positions
        # (own bin +- same-tile neighbours) form a contiguous local range.
        sloc = sseg[lo:hi]
        windows = []
        w_mats = []
        for wi in range(_ceil(MHALF, WIN) // WIN):
            blo = MHALF * h + WIN * wi
            bhi = min(blo + WIN, MHALF * (h + 1))
            bw = bhi - blo
            pl = int(np.searchsorted(sloc, max(blo - 1, MHALF * h)))
            ph = int(np.searchsorted(sloc, min(bhi + 1, MHALF * (h + 1))))
            if ph <= pl:
                continue  # window fully empty: its bins are never gathered
            c0b, c1b = pl // 128, (ph - 1) // 128
            blocks = list(range(c0b, c1b + 1))
            mats = []
            bins = np.arange(blo, bhi)
            for c in blocks:
                g0 = 128 * c
                rows = np.arange(g0, g0 + 128)
                sigma = np.full(128, -10**6, np.int64)
                valid = rows < n
                sigma[valid] = sloc[rows[valid]]
                diff = sigma[:, None] - bins[None, :]
                same_tile = (sigma[:, None] // N_FIBRES) == (bins[None, :] // N_FIBRES)
                W = (diff == 0).astype(np.float32)
                W += ((np.abs(diff) == 1) & same_tile).astype(np.float32) * etap
                Wp = np.zeros((128, WIN), np.float32)
                Wp[:, :bw] = W
                mats.append(Wp.astype(ml_dtypes.bfloat16))
            windows.append({"wi": wi, "blo": blo, "bw": bw, "blocks": blocks})
            w_mats.extend(mats)

        halves.append({
            "n": n, "n_p": n_p, "n_pb": n_pb, "nblk": nblk,
            "sort_idx": _wrap_idx(sort_flat),
            "windows": windows,
            "w_stack": np.concatenate([m for m in w_mats], axis=0)
            if w_mats else np.zeros((0, WIN), ml_dtypes.bfloat16),
        })

    out_idx = [_wrap_idx(seg[q * QUARTER:(q + 1) * QUARTER]) for q in range(4)]
    return halves, out_idx


def _build_program(halves):
    key = ("v6", tuple(h["n_p"] for h in halves),
           tuple((w["wi"], tuple(w["blocks"])) for h in halves
                 for w in h["windows"]))
    if key in _PROGRAM_CACHE:
        return _PROGRAM_CACHE[key]

    nc = bacc.Bacc("TRN2", target_bir_lowering=False, debug=False)

    flux_d = nc.dram_tensor("flux", [ROWS_PER_CORE, N_POS], F32,
                            kind="ExternalInput").ap()
    out_d = nc.dram_tensor("out", [ROWS_PER_CORE, N_POS], F32,
                           kind="ExternalOutput").ap()
    consts_d = nc.dram_tensor("consts", [P, 1], F32, kind="ExternalInput").ap()
    ident_d = nc.dram_tensor("ident", [P, P], BF16, kind="ExternalInput").ap()
    sort_d = [nc.dram_tensor(f"sort_idx{h}",
                             [P, _ceil(halves[h]["n_p"], 32) // 16], I16,
                             kind="ExternalInput").ap() for h in range(2)]
    oidx_d = [nc.dram_tensor(f"out_idx{q}", [P, _ceil(QUARTER, 32) // 16],
                             I16, kind="ExternalInput").ap() for q in range(4)]
    npairs = [sum(len(w["blocks"]) for w in halves[h]["windows"])
              for h in range(2)]
    w_d = [nc.dram_tensor(f"wmat{h}", [max(npairs[h], 1) * 128, WIN], BF16,
                          kind="ExternalInput").ap() for h in range(2)]

    Copy = mybir.ActivationFunctionType.Copy
    add = mybir.AluOpType.add

    with TileContext(nc) as tc:
        with (
            tc.tile_pool(name="pers", bufs=1) as pers,
            tc.tile_pool(name="big", bufs=2) as big_pool,
            tc.tile_pool(name="sh", bufs=2) as sh_pool,
            tc.tile_pool(name="fx", bufs=24) as fx_pool,
            tc.tile_pool(name="wm", bufs=8) as wm_pool,
            tc.tile_pool(name="stg", bufs=3) as stg_pool,
            tc.tile_pool(name="psT", bufs=4, space=MemorySpace.PSUM) as psT,
            tc.tile_pool(name="psB", bufs=4, space=MemorySpace.PSUM) as psB,
        ):
            sort_t = []
            for h in range(2):
                st = pers.tile([P, _ceil(halves[h]["n_p"], 32) // 16], I16,
                               tag=f"si{h}", name=f"sort{h}")
                nc.sync.dma_start(st[:], sort_d[h])
                sort_t.append(st)
            oidx_t = []
            for q in range(4):
                ot = pers.tile([P, _ceil(QUARTER, 32) // 16], I16,
                               tag=f"oi{q}", name=f"oidx{q}")
                nc.sync.dma_start(ot[:], oidx_d[q])
                oidx_t.append(ot)
            consts_t = pers.tile([P, 1], F32, tag="consts")
            nc.sync.dma_start(consts_t[:], consts_d)
            c0_ap = consts_t[:, 0:1]
            ident_t = pers.tile([P, P], BF16, tag="ident")
            nc.sync.dma_start(ident_t[:], ident_d)

            # ---- load + pack flux into F [P, N_POS, D] bf16 ----
            F = big_pool.tile([P, N_POS * D], BF16, tag="BIG", name="F")
            Fv = F[:].rearrange("p (j k) -> p j k", k=D)
            for k in range(D):
                for q in range(4):
                    fin = stg_pool.tile([P, INQ], F32, tag="STG",
                                        name=f"fin{k}{q}")
                    nc.sync.dma_start(
                        fin[:], flux_d[P * k:P * (k + 1),
                                       INQ * q:INQ * (q + 1)])
                    dst = Fv[:, INQ * q:INQ * (q + 1), k:k + 1].squeeze(2)
                    if (k * 4 + q) % 2 == 0:
                        nc.scalar.activation(dst, fin[:], Copy)
                    else:
                        nc.vector.tensor_scalar(dst, fin[:], 0.0, None, add)

            # ---- S tiles + pad-block zeroing ----
            S = []
            for h in range(2):
                n_p, n_pb = halves[h]["n_p"], halves[h]["n_pb"]
                t = sh_pool.tile([P, n_pb * D], BF16, tag="SH", name=f"S{h}")
                if n_pb > n_p:
                    nc.vector.memset(t[:, n_p * D:], 0.0)
                S.append(t)

            M = big_pool.tile([P, N_POS * D], BF16, tag="BIG", name="M")
            Mv = M[:].rearrange("p (j k) -> p j k", k=D)

            def g1(h):
                nc.gpsimd.ap_gather(S[h][:, :halves[h]["n_p"] * D], F[:],
                                    sort_t[h][:], channels=P, num_elems=N_POS,
                                    d=D, num_idxs=halves[h]["n_p"])

            def mid(h):
                hh = halves[h]
                Sv = S[h][:].rearrange("p (j k) -> p j k", k=D)
                fx = {}
                pair = 0
                for w in hh["windows"]:
                    for c in w["blocks"]:
                        if c not in fx:
                            fx[c] = []
                            for k in range(D):
                                lane = Sv[:, 128 * c:128 * (c + 1),
                                          k:k + 1].squeeze(2)
                                pt = psT.tile([P, P], BF16, tag="T",
                                              name=f"pt{h}_{c}_{k}")
                                nc.tensor.transpose(pt[:], lane, ident_t[:])
                                fxt = fx_pool.tile([P, P], BF16, tag="FX",
                                                   name=f"fx{h}_{c}_{k}")
                                nc.scalar.activation(fxt[:], pt[:], Copy)
                                fx[c].append(fxt)
                    wts = []
                    for c in w["blocks"]:
                        wt = wm_pool.tile([P, WIN], BF16, tag="WM",
                                          name=f"wm{h}_{w['wi']}_{c}")
                        nc.sync.dma_start(
                            wt[:], w_d[h][pair * 128:(pair + 1) * 128, :])
                        wts.append(wt)
                        pair += 1
                    bw = w["bw"]
                    for k in range(D):
                        pb = psB.tile([P, bw], F32, tag="B",
                                      name=f"pb{h}_{w['wi']}_{k}")
                        for ci, c in enumerate(w["blocks"]):
                            nc.tensor.matmul(
                                pb[:], fx[c][k][:], wts[ci][:, :bw],
                                start=(ci == 0),
                                stop=(ci == len(w["blocks"]) - 1))
                        mdst = Mv[:, w["blo"]:w["blo"] + bw,
                                  k:k + 1].squeeze(2)
                        nc.vector.tensor_scalar(mdst, pb[:], 0.0, None, add)

            def g3(q):
                Ot = sh_pool.tile([P, QUARTER * D], BF16, tag="SH",
                                  name=f"O{q}")
                nc.gpsimd.ap_gather(Ot[:], M[:], oidx_t[q][:], channels=P,
                                    num_elems=N_POS, d=D, num_idxs=QUARTER)
                Ov = Ot[:].rearrange("p (j k) -> p j k", k=D)
                for k in range(D):
                    ostg = stg_pool.tile([P, QUARTER], F32, tag="STG",
                                         name=f"ostg{q}{k}")
                    lane = Ov[:, :, k:k + 1].squeeze(2)
                    if k % 2 == 0:
                        nc.scalar.activation(ostg[:], lane, Copy, scale=c0_ap)
                    else:
                        nc.vector.tensor_scalar(ostg[:], lane, c0_ap, None,
                                                mybir.AluOpType.mult)
                    nc.sync.dma_start(
                        out_d[P * k:P * (k + 1),
                              QUARTER * q:QUARTER * (q + 1)], ostg[:])

            g1(0)
            g1(1)
            mid(0)
            mid(1)
            for q in range(4):
                g3(q)

    nc.compile()
    _PROGRAM_CACHE[key] = nc
    return nc


def _run(inputs, trace=False, trace_kwargs=None):
    flux = np.ascontiguousarray(np.asarray(inputs["flux"], dtype=np.float32))
    eta = float(np.asarray(inputs["eta"]).reshape(-1)[0])
    tile_idx = np.asarray(inputs["tile_idx"]).astype(np.int64).reshape(-1)
    fib_idx = np.asarray(inputs["fib_idx"]).astype(np.int64).reshape(-1)

    assert flux.shape == (N_WAVE, N_POS), flux.shape
    assert tile_idx.shape == (N_POS,) and fib_idx.shape == (N_POS,)

    c0 = np.float32(1.0 - 2.0 * eta)
    etap = float(np.float32(eta / float(c0)))
    halves, out_idx = _host_precompute(tile_idx, fib_idx, etap)

    nc = _build_program(halves)

    shared = {
        "consts": np.full((P, 1), c0, np.float32),
        "ident": np.eye(P, dtype=ml_dtypes.bfloat16),
    }
    for h in range(2):
        shared[f"sort_idx{h}"] = halves[h]["sort_idx"]
        ws = halves[h]["w_stack"]
        npair = max(ws.shape[0] // 128, 1)
        buf = np.zeros((npair * 128, WIN), ml_dtypes.bfloat16)
        buf[:ws.shape[0]] = ws
        shared[f"wmat{h}"] = buf
    for q in range(4):
        shared[f"out_idx{q}"] = out_idx[q]

    in_maps = []
    for c in range(N_CORES):
        m = dict(shared)
        m["flux"] = np.ascontiguousarray(
            flux[c * ROWS_PER_CORE:(c + 1) * ROWS_PER_CORE])
        in_maps.append(m)

    kwargs = {}
    if trace:
        kwargs["trace"] = True
        if trace_kwargs:
            kwargs["trace_kwargs"] = trace_kwargs
    res = run_bass_kernel_spmd(nc, in_maps, core_ids=list(range(N_CORES)),
                               **kwargs)
    out = np.concatenate([r["out"] for r in res.results], axis=0)
    return out, res


def kernel(**inputs) -> np.ndarray:
    out, _ = _run(inputs, trace=False)
    return out


# revision 19
# speedup vs baseline: 4.8086x; 1.0538x over previous
"""Trainium2 Bass kernel for nn_CrossTalk, v6: gathers + tensor-engine middle.

Full (unsharded) inputs in, full output out; wavelength axis sharded over 8
NeuronCores (512 rows/core). Per core the 512 rows ride as FOUR bf16 lanes
interleaved along the free axis ([128, 7200, 4]) so each gpsimd ap_gather
index moves all four row-chunks at once (~24-30ns/idx regardless of d).

Pipeline per core:
  G1h  gpsimd.ap_gather       : sort flux columns by segment id (2 halves,
                                split at the tile-6 boundary)
  PE   transpose + matmul     : per 128-bin window, per lane: accumulate
                                psum[wl, bins] = sum_blocks fluxT_ck @ W_cw
                                where W encodes BOTH the segment-sum one-hot
                                AND the tridiagonal crosstalk band (1 on the
                                bin, eta/c0 on same-tile neighbours) in f32
  DVE  tensor_scalar (1-input): psum -> M bf16 interleaved (strided copy)
  G3q  gpsimd.ap_gather       : gather M back to original positions, 4
                                quarter-gathers overlapped with output
                                conversion + DMA

The middle stage runs entirely on PE + Scalar + DVE-1-input ops, all of
which are measured contention-free against gpsimd gathers (unlike 2-input
DVE ops, which stall them ~1:1). Scan/segmented-sum DVE work is gone: the
matmul does the binning in f32 PSUM.

Precision: bf16 data path with f32 accumulation; rel err vs f32 reference
~6e-3 (harness gate 2e-2).
"""

import os
import sys

import numpy as np

for _p in ("/opt/trn_rl_repo", "/root/.axon_site/_ro/trn_rl_repo"):
    if os.path.isdir(_p) and _p not in sys.path:
        sys.path.insert(0, _p)

import ml_dtypes  # noqa: E402

import concourse.bacc as bacc  # noqa: E402
import concourse.mybir as mybir  # noqa: E402
from concourse.bass import MemorySpace  # noqa: E402
from concourse.tile import TileContext  # noqa: E402
from concourse.bass_utils import run_bass_kernel_spmd  # noqa: E402

N_WAVE = 4096
N_TILES = 12
N_FIBRES = 600
N_POS = N_TILES * N_FIBRES          # 7200
N_CORES = 8
ROWS_PER_CORE = N_WAVE // N_CORES   # 512
P = 128
D = 4                               # row-chunks per gather payload
HALF_TILES = N_TILES // 2
MHALF = HALF_TILES * N_FIBRES       # 3600 bins per half
WIN = 128                           # bin-window width for the PE stage
G1_SUB = 1792                       # first sub-gather size per half (14 blks)
G3_COLS = [(0, 1800), (1800, 1800), (3600, 1800), (5400, 900), (6300, 900)]
INQ = N_POS // 6                    # input load chunk (cols)

F32 = mybir.dt.float32
BF16 = mybir.dt.bfloat16
I16 = mybir.dt.int16

_PROGRAM_CACHE = {}


def _ceil(x, m):
    return (x + m - 1) // m * m


def _wrap_idx(flat):
    flat = np.asarray(flat, np.int64)
    padded = _ceil(flat.size, 32)
    if padded != flat.size:
        flat = np.concatenate([flat, np.zeros(padded - flat.size, np.int64)])
    w = flat.reshape(padded // 16, 16).T.astype(np.int16)
    return np.tile(w, (P // 16, 1))


def _host_precompute(tile_idx, fib_idx, etap):
    seg = tile_idx.astype(np.int64) * N_FIBRES + fib_idx.astype(np.int64)
    order = np.argsort(seg, kind="stable")
    sseg = seg[order]
    o6 = int(np.searchsorted(sseg, MHALF))

    halves = []
    for h in range(2):
        lo, hi = (0, o6) if h == 0 else (o6, N_POS)
        n = hi - lo
        n_p = _ceil(n, 16)      # gather count (mult of 4; 16 for wrap ease)
        n_pb = _ceil(n_p, 128)  # S width in blocks of 128
        nblk = n_pb // 128

        sort_flat = np.zeros(n_p, np.int64)
        sort_flat[:n] = order[lo:hi]
        subs = [(0, G1_SUB), (G1_SUB, n_p - G1_SUB)]

        # windows of WIN bins; per window the contributing sorted positions
        # (own bin +- same-tile neighbours) form a contiguous local range.
        sloc = sseg[lo:hi]
        windows = []
        w_mats = []
        for wi in range(_ceil(MHALF, WIN) // WIN):
            blo = MHALF * h + WIN * wi
            bhi = min(blo + WIN, MHALF * (h + 1))
            bw = bhi - blo
            pl = int(np.searchsorted(sloc, max(blo - 1, MHALF * h)))
            ph = int(np.searchsorted(sloc, min(bhi + 1, MHALF * (h + 1))))
            if ph <= pl:
                continue  # window fully empty: its bins are never gathered
            c0b, c1b = pl // 128, (ph - 1) // 128
            blocks = list(range(c0b, c1b + 1))
            mats = []
            bins = np.arange(blo, bhi)
            for c in blocks:
                g0 = 128 * c
                rows = np.arange(g0, g0 + 128)
                sigma = np.full(128, -10**6, np.int64)
                valid = rows < n
                sigma[valid] = sloc[rows[valid]]
                diff = sigma[:, None] - bins[None, :]
                same_tile = (sigma[:, None] // N_FIBRES) == (bins[None, :] // N_FIBRES)
                W = (diff == 0).astype(np.float32)
                W += ((np.abs(diff) == 1) & same_tile).astype(np.float32) * etap
                Wp = np.zeros((128, WIN), np.float32)
                Wp[:, :bw] = W
                mats.append(Wp.astype(ml_dtypes.bfloat16))
            windows.append({"wi": wi, "blo": blo, "bw": bw, "blocks": blocks})
            w_mats.extend(mats)

        halves.append({
            "n": n, "n_p": n_p, "n_pb": n_pb, "nblk": nblk,
            "subs": subs,
            "sort_idx": [_wrap_idx(sort_flat[s0:s0 + sn]) for s0, sn in subs],
            "windows": windows,
            "w_stack": np.concatenate([m for m in w_mats], axis=0)
            if w_mats else np.zeros((0, WIN), ml_dtypes.bfloat16),
        })

    out_idx = [_wrap_idx(seg[c:c + ncol]) for c, ncol in G3_COLS]
    return halves, out_idx


def _build_program(halves):
    key = ("v7", tuple(h["n_p"] for h in halves),
           tuple((w["wi"], tuple(w["blocks"])) for h in halves
                 for w in h["windows"]))
    if key in _PROGRAM_CACHE:
        return _PROGRAM_CACHE[key]

    nc = bacc.Bacc("TRN2", target_bir_lowering=False, debug=False)

    flux_d = nc.dram_tensor("flux", [ROWS_PER_CORE, N_POS], F32,
                            kind="ExternalInput").ap()
    out_d = nc.dram_tensor("out", [ROWS_PER_CORE, N_POS], F32,
                           kind="ExternalOutput").ap()
    consts_d = nc.dram_tensor("consts", [P, 1], F32, kind="ExternalInput").ap()
    ident_d = nc.dram_tensor("ident", [P, P], BF16, kind="ExternalInput").ap()
    sort_d = [[nc.dram_tensor(f"sort_idx{h}_{si}",
                              [P, _ceil(sn, 32) // 16], I16,
                              kind="ExternalInput").ap()
               for si, (s0, sn) in enumerate(halves[h]["subs"])]
              for h in range(2)]
    oidx_d = [nc.dram_tensor(f"out_idx{q}", [P, _ceil(ncol, 32) // 16],
                             I16, kind="ExternalInput").ap()
              for q, (c, ncol) in enumerate(G3_COLS)]
    npairs = [sum(len(w["blocks"]) for w in halves[h]["windows"])
              for h in range(2)]
    w_d = [nc.dram_tensor(f"wmat{h}", [max(npairs[h], 1) * 128, WIN], BF16,
                          kind="ExternalInput").ap() for h in range(2)]

    Copy = mybir.ActivationFunctionType.Copy
    add = mybir.AluOpType.add

    with TileContext(nc) as tc:
        with (
            tc.tile_pool(name="pers", bufs=1) as pers,
            tc.tile_pool(name="big", bufs=2) as big_pool,
            tc.tile_pool(name="sh", bufs=2) as sh_pool,
            tc.tile_pool(name="fx", bufs=20) as fx_pool,
            tc.tile_pool(name="wm", bufs=6) as wm_pool,
            tc.tile_pool(name="stg", bufs=4) as stg_pool,
            tc.tile_pool(name="psT", bufs=4, space=MemorySpace.PSUM) as psT,
            tc.tile_pool(name="psB", bufs=4, space=MemorySpace.PSUM) as psB,
        ):
            sort_t = []
            for h in range(2):
                sts = []
                for si, (s0, sn) in enumerate(halves[h]["subs"]):
                    st = pers.tile([P, _ceil(sn, 32) // 16], I16,
                                   tag=f"si{h}_{si}", name=f"sort{h}_{si}")
                    nc.sync.dma_start(st[:], sort_d[h][si])
                    sts.append(st)
                sort_t.append(sts)
            oidx_t = []
            for q, (c, ncol) in enumerate(G3_COLS):
                ot = pers.tile([P, _ceil(ncol, 32) // 16], I16,
                               tag=f"oi{q}", name=f"oidx{q}")
                nc.sync.dma_start(ot[:], oidx_d[q])
                oidx_t.append(ot)
            consts_t = pers.tile([P, 1], F32, tag="consts")
            nc.sync.dma_start(consts_t[:], consts_d)
            c0_ap = consts_t[:, 0:1]
            ident_t = pers.tile([P, P], BF16, tag="ident")
            nc.sync.dma_start(ident_t[:], ident_d)

            # ---- load + pack flux into F [P, N_POS, D] bf16 ----
            F = big_pool.tile([P, N_POS * D], BF16, tag="BIG", name="F")
            Fv = F[:].rearrange("p (j k) -> p j k", k=D)
            for k in range(D):
                for q in range(N_POS // INQ):
                    fin = stg_pool.tile([P, INQ], F32, tag="STG",
                                        name=f"fin{k}{q}")
                    nc.sync.dma_start(
                        fin[:], flux_d[P * k:P * (k + 1),
                                       INQ * q:INQ * (q + 1)])
                    dst = Fv[:, INQ * q:INQ * (q + 1), k:k + 1].squeeze(2)
                    # vector is ~2x faster on the strided bf16 write;
                    # give it 2/3 of the conversions.
                    if (k * (N_POS // INQ) + q) % 3 == 0:
                        nc.scalar.activation(dst, fin[:], Copy)
                    else:
                        nc.vector.tensor_scalar(dst, fin[:], 0.0, None, add)

            # ---- S tiles + pad-block zeroing ----
            S = []
            for h in range(2):
                n_p, n_pb = halves[h]["n_p"], halves[h]["n_pb"]
                t = sh_pool.tile([P, n_pb * D], BF16, tag="SH", name=f"S{h}")
                if n_pb > n_p:
                    nc.vector.memset(t[:, n_p * D:], 0.0)
                S.append(t)

            M = big_pool.tile([P, N_POS * D], BF16, tag="BIG", name="M")
            Mv = M[:].rearrange("p (j k) -> p j k", k=D)

            def g1(h, si):
                s0, sn = halves[h]["subs"][si]
                nc.gpsimd.ap_gather(S[h][:, s0 * D:(s0 + sn) * D], F[:],
                                    sort_t[h][si][:], channels=P,
                                    num_elems=N_POS, d=D, num_idxs=sn)

            def mid(h):
                hh = halves[h]
                Sv = S[h][:].rearrange("p (j k) -> p j k", k=D)
                fx = {}
                pair = 0
                for w in hh["windows"]:
                    for c in w["blocks"]:
                        if c not in fx:
                            fx[c] = []
                            for k in range(D):
                                lane = Sv[:, 128 * c:128 * (c + 1),
                                          k:k + 1].squeeze(2)
                                pt = psT.tile([P, P], BF16, tag="T",
                                              name=f"pt{h}_{c}_{k}")
                                nc.tensor.transpose(pt[:], lane, ident_t[:])
                                fxt = fx_pool.tile([P, P], BF16, tag="FX",
                                                   name=f"fx{h}_{c}_{k}")
                                nc.scalar.activation(fxt[:], pt[:], Copy)
                                fx[c].append(fxt)
                    wts = []
                    for c in w["blocks"]:
                        wt = wm_pool.tile([P, WIN], BF16, tag="WM",
                                          name=f"wm{h}_{w['wi']}_{c}")
                        nc.sync.dma_start(
                            wt[:], w_d[h][pair * 128:(pair + 1) * 128, :])
                        wts.append(wt)
                        pair += 1
                    bw = w["bw"]
                    for k in range(D):
                        pb = psB.tile([P, bw], F32, tag="B",
                                      name=f"pb{h}_{w['wi']}_{k}")
                        for ci, c in enumerate(w["blocks"]):
                            nc.tensor.matmul(
                                pb[:], fx[c][k][:], wts[ci][:, :bw],
                                start=(ci == 0),
                                stop=(ci == len(w["blocks"]) - 1))
                        mdst = Mv[:, w["blo"]:w["blo"] + bw,
                                  k:k + 1].squeeze(2)
                        nc.vector.tensor_scalar(mdst, pb[:], 0.0, None, add)

            def g3(q):
                c, ncol = G3_COLS[q]
                Ot = sh_pool.tile([P, ncol * D], BF16, tag="SH",
                                  name=f"O{q}")
                nc.gpsimd.ap_gather(Ot[:], M[:], oidx_t[q][:], channels=P,
                                    num_elems=N_POS, d=D, num_idxs=ncol)
                Ov = Ot[:].rearrange("p (j k) -> p j k", k=D)
                for k in range(D):
                    ostg = stg_pool.tile([P, ncol], F32, tag="STG",
                                         name=f"ostg{q}{k}")
                    lane = Ov[:, :, k:k + 1].squeeze(2)
                    if k % 2 == 0:
                        nc.scalar.activation(ostg[:], lane, Copy, scale=c0_ap)
                    else:
                        nc.vector.tensor_scalar(ostg[:], lane, c0_ap, None,
                                                mybir.AluOpType.mult)
                    nc.sync.dma_start(
                        out_d[P * k:P * (k + 1), c:c + ncol], ostg[:])

            for h in range(2):
                for si in range(len(halves[h]["subs"])):
                    g1(h, si)
            mid(0)
            mid(1)
            for q in range(len(G3_COLS)):
                g3(q)

    nc.compile()
    _PROGRAM_CACHE[key] = nc
    return nc


def _run(inputs, trace=False, trace_kwargs=None):
    flux = np.ascontiguousarray(np.asarray(inputs["flux"], dtype=np.float32))
    eta = float(np.asarray(inputs["eta"]).reshape(-1)[0])
    tile_idx = np.asarray(inputs["tile_idx"]).astype(np.int64).reshape(-1)
    fib_idx = np.asarray(inputs["fib_idx"]).astype(np.int64).reshape(-1)

    assert flux.shape == (N_WAVE, N_POS), flux.shape
    assert tile_idx.shape == (N_POS,) and fib_idx.shape == (N_POS,)

    c0 = np.float32(1.0 - 2.0 * eta)
    etap = float(np.float32(eta / float(c0)))
    halves, out_idx = _host_precompute(tile_idx, fib_idx, etap)

    nc = _build_program(halves)

    shared = {
        "consts": np.full((P, 1), c0, np.float32),
        "ident": np.eye(P, dtype=ml_dtypes.bfloat16),
    }
    for h in range(2):
        for si in range(len(halves[h]["subs"])):
            shared[f"sort_idx{h}_{si}"] = halves[h]["sort_idx"][si]
        ws = halves[h]["w_stack"]
        npair = max(ws.shape[0] // 128, 1)
        buf = np.zeros((npair * 128, WIN), ml_dtypes.bfloat16)
        buf[:ws.shape[0]] = ws
        shared[f"wmat{h}"] = buf
    for q in range(len(G3_COLS)):
        shared[f"out_idx{q}"] = out_idx[q]

    in_maps = []
    for c in range(N_CORES):
        m = dict(shared)
        m["flux"] = np.ascontiguousarray(
            flux[c * ROWS_PER_CORE:(c + 1) * ROWS_PER_CORE])
        in_maps.append(m)

    kwargs = {}
    if trace:
        kwargs["trace"] = True
        if trace_kwargs:
            kwargs["trace_kwargs"] = trace_kwargs
    res = run_bass_kernel_spmd(nc, in_maps, core_ids=list(range(N_CORES)),
                               **kwargs)
    out = np.concatenate([r["out"] for r in res.results], axis=0)
    return out, res


def kernel(**inputs) -> np.ndarray:
    out, _ = _run(inputs, trace=False)
    return out
